# revision 1
# baseline (speedup 1.0000x reference)
"""GNO message-passing kernel for Trainium2 (8 NeuronCores, edge-parallel).

Math (matches the reference):
    h  = relu(relu(relu(ea@W1+b1)@W2+b2)@W3+b3)
    w  = (h@W4+b4).reshape(E,16,16)
    msg= einsum('ei,eio->eo', x[src], w)
    agg= segment_mean(msg, dst, N)
    out= x@root + agg + bias

Strategy:
  - Edges are split into 8 contiguous shards (one per core).  Each shard is
    sorted by dst and runs of equal dst are padded so no run crosses a
    128-edge group boundary.  Hence each dst's partial sum is produced by
    exactly one matmul slot on exactly one core -> scatter writes never
    collide across DMAs.
  - Per 512-edge tile on-device: bf16 MLP on TensorE (channel-major,
    bias-as-extra-row for layer 4), indirect-DMA gather of x[src], einsum
    on VectorE (broadcast multiply + strided reduce), host-precomputed
    one-hot segment matrix matmul to collapse equal-dst rows, indirect
    scatter-add into a per-core accumulator table [N+128, 17]
    (16 msg sums + count).
  - x@root+bias computed on-device (node-sliced across cores).
  - Host: sum the 8 accumulator tables, divide by counts, add root part.
"""

import math
import numpy as np
import ml_dtypes

import concourse.bass as bass
import concourse.bacc as bacc
import concourse.mybir as mybir
import concourse.tile as tile
from concourse.bass_utils import run_bass_kernel_spmd

BF16 = ml_dtypes.bfloat16

N_NODES = 50000
N_EDGES = 800000
N_CORES = 8
ETILE = 512
P = 128
NSLICE = N_NODES // N_CORES  # 6250 nodes per core for x@root


# ----------------------------------------------------------------- host prep

def _pack_shard(src, dst, attr, n_tiles):
    """Sort a shard's edges by dst and pad so no equal-dst run crosses a
    128-edge group boundary. Returns (attrT bf16 [8,Ep], meta int32
    [T,128,8], segm bf16 [T,128,512]). meta cols 0:4 = src idx per group,
    cols 4:8 = scatter row per (group, slot)."""
    E0 = len(dst)
    order = np.argsort(dst, kind="stable")
    src, dst, attr = src[order], dst[order], attr[order]

    # run lengths of equal dst
    bound = np.flatnonzero(np.diff(dst)) + 1
    starts = np.concatenate([[0], bound])
    lens = np.diff(np.concatenate([starts, [E0]]))
    assert lens.max() <= P, f"in-degree {lens.max()} > 128 unsupported"

    # greedy: new start of each run, padding to group boundary when crossing
    new_starts = np.empty(len(lens), np.int64)
    fill = 0
    pos = 0
    ll = lens.tolist()
    for i, l in enumerate(ll):
        if fill + l > P:
            pos += P - fill
            fill = 0
        new_starts[i] = pos
        pos += l
        fill += l
        if fill == P:
            fill = 0
    Ep = n_tiles * ETILE
    assert pos <= Ep, f"padded edges {pos} exceed capacity {Ep}"

    # expand to per-edge new positions
    new_pos = np.repeat(new_starts, lens) + (np.arange(E0) - np.repeat(starts, lens))
    src_p = np.zeros(Ep, np.int64)
    dst_p = np.full(Ep, N_NODES, np.int64)
    attr_p = np.zeros((Ep, 8), np.float32)
    src_p[new_pos] = src
    dst_p[new_pos] = dst
    attr_p[new_pos] = attr

    NG = Ep // P
    dg = dst_p.reshape(NG, P)
    first = np.ones((NG, P), bool)
    first[:, 1:] = dg[:, 1:] != dg[:, :-1]
    rank = np.cumsum(first, axis=1) - 1  # slot of each edge within its group

    gi = np.arange(NG)[:, None]
    pi = np.broadcast_to(np.arange(P)[None, :], (NG, P))
    segm = np.zeros((NG, P, P), BF16)
    segm[gi, pi, rank] = BF16(1.0)
    segm = segm.reshape(n_tiles, 4, P, P).transpose(0, 2, 1, 3).reshape(n_tiles, P, 4 * P)
    segm = np.ascontiguousarray(segm)

    # node owning each (group, slot); >= N_NODES marks unused/pad slots
    ie = np.full((NG, P), N_NODES, np.int64)
    ie[gi, rank] = dg
    # node_of[t, s, g] matches the dense accum layout [T, 128(slot), 4(g), 17]
    node_of = np.ascontiguousarray(
        ie.reshape(n_tiles, 4, P).transpose(0, 2, 1)).astype(np.int64)

    meta = np.ascontiguousarray(
        src_p.reshape(n_tiles, 4, P).transpose(0, 2, 1)).astype(np.int32)

    attrT = np.ascontiguousarray(attr_p.T).astype(BF16)
    return attrT, meta, segm, node_of


def _prep_inputs(x, edge_index, edge_attr, W1, b1, W2, b2, W3, b3, W4, b4,
                 root, bias):
    src_all = np.asarray(edge_index[0], np.int64)
    dst_all = np.asarray(edge_index[1], np.int64)
    attr_all = np.asarray(edge_attr, np.float32)
    Esh = N_EDGES // N_CORES

    shards = []
    t_needed = 0
    for k in range(N_CORES):
        sl = slice(k * Esh, (k + 1) * Esh)
        dst = dst_all[sl]
        # padded length for this shard (same greedy as _pack_shard)
        order = np.argsort(dst, kind="stable")
        ds = dst[order]
        bound = np.flatnonzero(np.diff(ds)) + 1
        lens = np.diff(np.concatenate([[0], bound, [Esh]]))
        fill = pos = 0
        for l in lens.tolist():
            if fill + l > P:
                pos += P - fill
                fill = 0
            pos += l
            fill += l
            if fill == P:
                fill = 0
        t_needed = max(t_needed, math.ceil(pos / ETILE))
        shards.append((src_all[sl], dst, attr_all[sl]))
    T = t_needed

    # weights, channel-major layouts
    W4p = np.asarray(W4, np.float32).reshape(100, 16, 16).transpose(0, 2, 1).reshape(100, 256)
    b4p = np.asarray(b4, np.float32).reshape(16, 16).T.reshape(256)
    W4a = np.concatenate([W4p, b4p[None, :]], axis=0).astype(BF16)  # [101,256]
    roota = np.concatenate([np.asarray(root, np.float32),
                            np.asarray(bias, np.float32)[None, :]], axis=0).astype(BF16)
    # widen W3 by a zero output column whose bias is 1.0: after ReLU the
    # extra channel is the constant 1 row that feeds W4a's bias row
    W3a = np.concatenate([np.asarray(W3, np.float32),
                          np.zeros((100, 1), np.float32)], axis=1).astype(BF16)
    b3a = np.concatenate([np.asarray(b3, np.float32),
                          np.ones(1, np.float32)]).reshape(101, 1)
    const = {
        "W1": np.asarray(W1, np.float32).astype(BF16),
        "W2": np.asarray(W2, np.float32).astype(BF16),
        "W3": W3a,
        "W4a": W4a,
        "b1": np.asarray(b1, np.float32).reshape(100, 1),
        "b2": np.asarray(b2, np.float32).reshape(100, 1),
        "b3": b3a,
        "roota": roota,
        "xfull": np.asarray(x, np.float32),
    }

    in_maps = []
    node_maps = []
    for k in range(N_CORES):
        attrT, meta, segm, node_of = _pack_shard(*shards[k], T)
        node_maps.append(node_of)
        xsl = np.asarray(x[k * NSLICE:(k + 1) * NSLICE], np.float32).T
        xslT = np.ascontiguousarray(
            np.concatenate([xsl, np.ones((1, NSLICE), np.float32)], axis=0)
        ).astype(BF16)  # [17, NSLICE] with ones row for the bias
        in_maps.append(dict(const, attrT=attrT, meta=meta, segm=segm, xslT=xslT))
    return in_maps, node_maps, T


# ------------------------------------------------------------ device program

_PROG_CACHE = {}


def build_program(T, n_nodes=N_NODES, nslice=NSLICE):
    key = (T, n_nodes, nslice)
    if key in _PROG_CACHE:
        return _PROG_CACHE[key]

    f32, bf16, i32 = mybir.dt.float32, mybir.dt.bfloat16, mybir.dt.int32
    Ep = T * ETILE
    trows = n_nodes + P

    nc = bacc.Bacc(None, target_bir_lowering=False, debug=True)
    attrT = nc.dram_tensor("attrT", [8, Ep], bf16, kind="ExternalInput")
    meta = nc.dram_tensor("meta", [T, P, 4], i32, kind="ExternalInput")
    segm = nc.dram_tensor("segm", [T, P, 4 * P], bf16, kind="ExternalInput")
    xfull = nc.dram_tensor("xfull", [n_nodes, 16], f32, kind="ExternalInput")
    xslT = nc.dram_tensor("xslT", [17, nslice], bf16, kind="ExternalInput")
    W1 = nc.dram_tensor("W1", [8, 100], bf16, kind="ExternalInput")
    W2 = nc.dram_tensor("W2", [100, 100], bf16, kind="ExternalInput")
    W3 = nc.dram_tensor("W3", [100, 101], bf16, kind="ExternalInput")
    W4a = nc.dram_tensor("W4a", [101, 256], bf16, kind="ExternalInput")
    b1 = nc.dram_tensor("b1", [100, 1], f32, kind="ExternalInput")
    b2 = nc.dram_tensor("b2", [100, 1], f32, kind="ExternalInput")
    b3 = nc.dram_tensor("b3", [101, 1], f32, kind="ExternalInput")
    roota = nc.dram_tensor("roota", [17, 16], bf16, kind="ExternalInput")
    accum = nc.dram_tensor("accum", [T, P, 4 * 17], f32, kind="ExternalOutput")
    rootp = nc.dram_tensor("rootp", [nslice, 16], f32, kind="ExternalOutput")

    AT = mybir.ActivationFunctionType
    AX = mybir.AxisListType
    OP = mybir.AluOpType

    with tile.TileContext(nc) as tc, \
         nc.allow_low_precision(reason="bf16 intermediates, fp32 accumulation"):
        with tc.tile_pool(name="consts", bufs=1) as cp, \
             tc.tile_pool(name="work", bufs=3) as wp, \
             tc.tile_pool(name="small", bufs=8) as sp, \
             tc.tile_pool(name="psmlp", bufs=2, space="PSUM") as pm, \
             tc.tile_pool(name="psw", bufs=3, space="PSUM") as pw, \
             tc.tile_pool(name="psagg", bufs=2, space="PSUM") as pa:

            W1sb = cp.tile([8, 100], bf16)
            W2sb = cp.tile([100, 100], bf16)
            W3sb = cp.tile([100, 101], bf16)
            W4sb = cp.tile([101, 256], bf16)
            b1sb = cp.tile([100, 1], f32)
            b2sb = cp.tile([100, 1], f32)
            b3sb = cp.tile([101, 1], f32)
            rsb = cp.tile([17, 16], bf16)
            for t_sb, t_dr in ((W1sb, W1), (W2sb, W2), (W3sb, W3), (W4sb, W4a),
                               (b1sb, b1), (b2sb, b2), (b3sb, b3), (rsb, roota)):
                nc.sync.dma_start(t_sb[:], t_dr[:])

            for t in range(T):
                a_sb = wp.tile([8, ETILE], bf16, tag="attr")
                nc.sync.dma_start(a_sb[:], attrT[:, t * ETILE:(t + 1) * ETILE])
                m_sb = wp.tile([P, 4], i32, tag="meta")
                nc.sync.dma_start(m_sb[:], meta[t])
                s_sb = wp.tile([P, 4 * P], bf16, tag="segm")
                nc.sync.dma_start(s_sb[:], segm[t])
                xg = wp.tile([P, 4, 16], f32, tag="xg")
                for g in range(4):
                    # HW DGE only supports one index per partition per DMA
                    nc.gpsimd.indirect_dma_start(
                        out=xg[:, g, :], out_offset=None, in_=xfull[:],
                        in_offset=bass.IndirectOffsetOnAxis(ap=m_sb[:, g:g + 1], axis=0))

                ps1 = pm.tile([100, ETILE], f32, tag="mlp")
                nc.tensor.matmul(ps1[:], lhsT=W1sb[:], rhs=a_sb[:], start=True, stop=True)
                h1 = wp.tile([100, ETILE], bf16, tag="h1")
                nc.scalar.activation(h1[:], ps1[:], AT.Relu, bias=b1sb[:, 0:1])
                ps2 = pm.tile([100, ETILE], f32, tag="mlp")
                nc.tensor.matmul(ps2[:], lhsT=W2sb[:], rhs=h1[:], start=True, stop=True)
                h2 = wp.tile([100, ETILE], bf16, tag="h2")
                nc.scalar.activation(h2[:], ps2[:], AT.Relu, bias=b2sb[:, 0:1])
                ps3 = pm.tile([101, ETILE], f32, tag="mlp")
                nc.tensor.matmul(ps3[:], lhsT=W3sb[:], rhs=h2[:], start=True, stop=True)
                h3 = wp.tile([101, ETILE], bf16, tag="h3")
                nc.scalar.activation(h3[:], ps3[:], AT.Relu, bias=b3sb[:, 0:1])

                scat = wp.tile([P, 4, 17], f32, tag="scat")
                mt = sp.tile([P, 4, 17], bf16, tag="msg")
                nc.gpsimd.memset(mt[:, :, 16:17], 1.0)
                for g in range(4):
                    wps = pw.tile([P, 256], f32, tag="w")
                    nc.tensor.matmul(wps[:], lhsT=h3[:, g * P:(g + 1) * P],
                                     rhs=W4sb[:], start=True, stop=True)
                    pr = sp.tile([P, 256], bf16, tag="prod")
                    nc.vector.tensor_tensor(
                        out=pr[:].rearrange("p (o i) -> p o i", i=16),
                        in0=wps[:].rearrange("p (o i) -> p o i", i=16),
                        in1=xg[:, g, :][:, None, :].to_broadcast([P, 16, 16]),
                        op=OP.mult)
                    nc.vector.reduce_sum(
                        out=mt[:, g, 0:16],
                        in_=pr[:].rearrange("p (o i) -> p o i", i=16), axis=AX.X)
                    ag = pa.tile([P, 17], f32, tag="agg")
                    nc.tensor.matmul(ag[:], lhsT=s_sb[:, g * P:(g + 1) * P],
                                     rhs=mt[:, g, :], start=True, stop=True)
                    nc.scalar.copy(scat[:, g, :], ag[:])
                # dense write: host redistributes rows by the packing map
                nc.sync.dma_start(accum[t], scat[:].rearrange("p a b -> p (a b)"))

            # x@root + bias for this core's node slice
            for c in range(math.ceil(nslice / P)):
                n0 = c * P
                w = min(P, nslice - n0)
                xt = wp.tile([17, P], bf16, tag="xt")
                nc.gpsimd.memset(xt[:], 0.0)
                nc.sync.dma_start(xt[:, :w], xslT[:, n0:n0 + w])
                rp = pa.tile([P, 16], f32, tag="agg")
                nc.tensor.matmul(rp[:], lhsT=xt[:], rhs=rsb[:], start=True, stop=True)
                ro = wp.tile([P, 16], f32, tag="ro")
                nc.scalar.copy(ro[:w, :], rp[:w, :])
                nc.sync.dma_start(rootp[n0:n0 + w, :], ro[:w, :])

    nc.compile()
    _PROG_CACHE[key] = nc
    return nc


# ------------------------------------------------------------------- driver

def _combine(results, node_maps, n_nodes):
    acc = np.zeros((n_nodes, 17), np.float64)
    rootparts = []
    for r, node_of in zip(results, node_maps):
        dense = np.asarray(r["accum"], np.float64).reshape(-1, 17)
        nodes = node_of.ravel()
        valid = nodes < n_nodes
        # each node occupies exactly one slot per core -> plain indexed add
        acc[nodes[valid]] += dense[valid]
        rootparts.append(np.asarray(r["rootp"], np.float32))
    agg = acc[:, :16] / np.maximum(acc[:, 16], 1.0)[:, None]
    return np.concatenate(rootparts, axis=0) + agg.astype(np.float32)


def _run(inputs, trace=False):
    in_maps, node_maps, T = _prep_inputs(**inputs)
    nc = build_program(T)
    res = run_bass_kernel_spmd(nc, in_maps, list(range(N_CORES)), trace=trace)
    out = _combine(res.results, node_maps, N_NODES)
    return out.astype(np.float32), res


def kernel(**inputs) -> np.ndarray:
    out, _ = _run(inputs, trace=False)
    return out



# revision 4
# speedup vs baseline: 5731.2475x; 5731.2475x over previous
"""GNO message-passing kernel for Trainium2 (8 NeuronCores, edge-parallel).

Math (matches the reference):
    h  = relu(relu(relu(ea@W1+b1)@W2+b2)@W3+b3)
    w  = (h@W4+b4).reshape(E,16,16)
    msg= einsum('ei,eio->eo', x[src], w)
    agg= segment_mean(msg, dst, N)
    out= x@root + agg + bias

Strategy (v2 — minimal host<->device I/O, on-device aggregation):
  - Edges are split into 8 contiguous shards (one per core) and sorted by
    dst.  Nodes are grouped into 128-node windows; each window's edges are
    packed into G groups of 128 (G uniform across windows/cores so all 8
    cores run one SPMD program).  Per window-tile the device runs the edge
    MLP (TensorE, bf16), an indirect-DMA gather of x[src], the per-edge
    einsum on VectorE, builds the segment one-hot ON DEVICE (is_equal vs an
    iota constant), and accumulates the window's 128 node sums in PSUM via
    G matmuls, then writes them contiguously into a per-core [N,16] table.
  - x is sharded: each core uploads its [N/8,16] slice; an AllGather
    produces the full gather table on device.  After the tile loop a
    ReduceScatter sums the 8 per-core tables and hands each core its node
    slice, which it finalizes (divide by host-precomputed 1/cnt, add
    x@root + bias) and writes as the only output.
  - Host work: sort/pack (vectorized numpy), bincount for 1/cnt, and a
    final concat of the 8 output slices.
"""

import math
import numpy as np
import ml_dtypes

import concourse.bass as bass
import concourse.bacc as bacc
import concourse.mybir as mybir
import concourse.tile as tile
from concourse.bass_utils import run_bass_kernel_spmd

BF16 = ml_dtypes.bfloat16

N_NODES = 50000
N_EDGES = 800000
N_CORES = 8
P = 128
NW = 392                    # 128-node windows (incl. padding windows)
N_PAD = NW * P              # 50176
NSLICE = N_PAD // N_CORES   # 6272 nodes per core
NCHUNK = NSLICE // P        # 49 finalize chunks per core
ESH = N_EDGES // N_CORES    # 100000 edges per core


# ----------------------------------------------------------------- host prep

def _pack_shard(src, dst, attr, G):
    """Sort by dst, pack each 128-node window's edges into G groups of 128.
    Returns attrT bf16 [8,Ep], msrc i32 [NW,128,G], slotv bf16 [NW,128,G]."""
    order = np.argsort(dst, kind="stable")
    src, dst, attr = src[order], dst[order], attr[order]
    w = dst >> 7
    cnt_w = np.bincount(w, minlength=NW)
    assert cnt_w.max() <= G * P
    starts = np.zeros(NW + 1, np.int64)
    starts[1:] = np.cumsum(cnt_w)
    pos = np.arange(len(dst)) - starts[w]
    WT = G * P
    flat = w * WT + pos
    Ep = NW * WT
    src_p = np.zeros(Ep, np.int32)
    slot_p = np.full(Ep, -1.0, np.float32)
    attr_p = np.zeros((Ep, 8), np.float32)
    src_p[flat] = src
    slot_p[flat] = dst & 127
    attr_p[flat] = attr
    msrc = np.ascontiguousarray(src_p.reshape(NW, G, P).transpose(0, 2, 1))
    slotv = np.ascontiguousarray(
        slot_p.reshape(NW, G, P).transpose(0, 2, 1)).astype(BF16)
    attrT = np.ascontiguousarray(attr_p.T).astype(BF16)
    return attrT, msrc, slotv


def _prep_inputs(x, edge_index, edge_attr, W1, b1, W2, b2, W3, b3, W4, b4,
                 root, bias):
    src_all = np.asarray(edge_index[0], np.int64).astype(np.int32)
    dst_all = np.asarray(edge_index[1], np.int64).astype(np.int32)
    attr_all = np.asarray(edge_attr, np.float32)

    # uniform G = max groups any (core, window) needs
    maxc = 0
    for k in range(N_CORES):
        d = dst_all[k * ESH:(k + 1) * ESH]
        maxc = max(maxc, np.bincount(d >> 7, minlength=NW).max())
    G = max(2, math.ceil(maxc / P))

    # weights, channel-major layouts (bias of W4 as an extra MLP channel)
    W4p = np.asarray(W4, np.float32).reshape(100, 16, 16).transpose(0, 2, 1).reshape(100, 256)
    b4p = np.asarray(b4, np.float32).reshape(16, 16).T.reshape(256)
    W4a = np.concatenate([W4p, b4p[None, :]], axis=0).astype(BF16)  # [101,256]
    roota = np.concatenate([np.asarray(root, np.float32),
                            np.asarray(bias, np.float32)[None, :]], axis=0).astype(BF16)
    W3a = np.concatenate([np.asarray(W3, np.float32),
                          np.zeros((100, 1), np.float32)], axis=1).astype(BF16)
    b3a = np.concatenate([np.asarray(b3, np.float32),
                          np.ones(1, np.float32)]).reshape(101, 1)
    iota = np.ascontiguousarray(
        np.tile(np.arange(P, dtype=np.float32), (P, G))).astype(BF16)  # [128, G*128]

    xp = np.zeros((N_PAD, 16), np.float32)
    xp[:N_NODES] = np.asarray(x, np.float32)
    xb = xp.astype(BF16)
    cnt = np.bincount(dst_all, minlength=N_PAD).astype(np.float32)
    recip = 1.0 / np.maximum(cnt, 1.0)

    const = {
        "W1": np.asarray(W1, np.float32).astype(BF16),
        "W2": np.asarray(W2, np.float32).astype(BF16),
        "W3": W3a,
        "W4a": W4a,
        "b1": np.asarray(b1, np.float32).reshape(100, 1),
        "b2": np.asarray(b2, np.float32).reshape(100, 1),
        "b3": b3a,
        "roota": roota,
        "iota": iota,
    }

    in_maps = []
    for k in range(N_CORES):
        sl = slice(k * ESH, (k + 1) * ESH)
        attrT, msrc, slotv = _pack_shard(src_all[sl], dst_all[sl], attr_all[sl], G)
        xsl = xb[k * NSLICE:(k + 1) * NSLICE]                  # [6272,16] bf16
        xslT = np.ascontiguousarray(
            np.concatenate([xsl.T.astype(np.float32),
                            np.ones((1, NSLICE), np.float32)], axis=0)).astype(BF16)
        recipT = np.ascontiguousarray(
            recip[k * NSLICE:(k + 1) * NSLICE].reshape(NCHUNK, P).T)  # [128,49]
        in_maps.append(dict(const, attrT=attrT, msrc=msrc, slotv=slotv,
                            x8=np.ascontiguousarray(xsl), xslT=xslT,
                            recipT=recipT))
    return in_maps, G


# ------------------------------------------------------------ device program

_PROG_CACHE = {}


def build_program(G):
    if G in _PROG_CACHE:
        return _PROG_CACHE[G]

    f32, bf16, i32 = mybir.dt.float32, mybir.dt.bfloat16, mybir.dt.int32
    WT = G * P
    Ep = NW * WT

    nc = bacc.Bacc(None, target_bir_lowering=False, debug=False, num_devices=N_CORES)
    attrT = nc.dram_tensor("attrT", [8, Ep], bf16, kind="ExternalInput")
    msrc = nc.dram_tensor("msrc", [NW, P, G], i32, kind="ExternalInput")
    slotv = nc.dram_tensor("slotv", [NW, P, G], bf16, kind="ExternalInput")
    x8 = nc.dram_tensor("x8", [NSLICE, 16], bf16, kind="ExternalInput")
    xslT = nc.dram_tensor("xslT", [17, NSLICE], bf16, kind="ExternalInput")
    recipT = nc.dram_tensor("recipT", [P, NCHUNK], f32, kind="ExternalInput")
    W1 = nc.dram_tensor("W1", [8, 100], bf16, kind="ExternalInput")
    W2 = nc.dram_tensor("W2", [100, 100], bf16, kind="ExternalInput")
    W3 = nc.dram_tensor("W3", [100, 101], bf16, kind="ExternalInput")
    W4a = nc.dram_tensor("W4a", [101, 256], bf16, kind="ExternalInput")
    b1 = nc.dram_tensor("b1", [100, 1], f32, kind="ExternalInput")
    b2 = nc.dram_tensor("b2", [100, 1], f32, kind="ExternalInput")
    b3 = nc.dram_tensor("b3", [101, 1], f32, kind="ExternalInput")
    roota = nc.dram_tensor("roota", [17, 16], bf16, kind="ExternalInput")
    iota = nc.dram_tensor("iota", [P, WT], bf16, kind="ExternalInput")
    out = nc.dram_tensor("out", [NSLICE, 16], f32, kind="ExternalOutput")

    x8i = nc.dram_tensor("x8i", [NSLICE, 16], bf16, kind="Internal")
    xga = nc.dram_tensor("xga", [N_PAD, 16], bf16, kind="Internal",
                         addr_space="Shared")
    table = nc.dram_tensor("table", [N_PAD, 16], f32, kind="Internal")
    rs_out = nc.dram_tensor("rs_out", [NSLICE, 16], f32, kind="Internal")

    AT = mybir.ActivationFunctionType
    AX = mybir.AxisListType
    OP = mybir.AluOpType
    RG = [list(range(N_CORES))]

    with tile.TileContext(nc) as tc, \
         nc.allow_low_precision(reason="bf16 intermediates, fp32 accumulation"):
        with tc.tile_pool(name="consts", bufs=1) as cp, \
             tc.tile_pool(name="work", bufs=3) as wp, \
             tc.tile_pool(name="psmlp", bufs=2, space="PSUM") as pm, \
             tc.tile_pool(name="psw", bufs=3, space="PSUM") as pw, \
             tc.tile_pool(name="psagg", bufs=2, space="PSUM") as pa:

            W1sb = cp.tile([8, 100], bf16)
            W2sb = cp.tile([100, 100], bf16)
            W3sb = cp.tile([100, 101], bf16)
            W4sb = cp.tile([101, 256], bf16)
            b1sb = cp.tile([100, 1], f32)
            b2sb = cp.tile([100, 1], f32)
            b3sb = cp.tile([101, 1], f32)
            rsb = cp.tile([17, 16], bf16)
            iosb = cp.tile([P, WT], bf16)
            xtsb = cp.tile([17, NSLICE], bf16)
            rcsb = cp.tile([P, NCHUNK], f32)
            for t_sb, t_dr in ((W1sb, W1), (W2sb, W2), (W3sb, W3), (W4sb, W4a),
                               (b1sb, b1), (b2sb, b2), (b3sb, b3), (rsb, roota),
                               (iosb, iota), (xtsb, xslT), (rcsb, recipT)):
                nc.sync.dma_start(t_sb[:], t_dr[:])

            # stage x slice -> internal dram, AllGather to full table
            xb_sb = cp.tile([P, NCHUNK, 16], bf16)
            nc.sync.dma_start(
                xb_sb[:], x8[:].rearrange("(c p) i -> p c i", p=P))
            nc.sync.dma_start(
                x8i[:].rearrange("(c p) i -> p c i", p=P), xb_sb[:])
            nc.gpsimd.collective_compute(
                "AllGather", OP.bypass, replica_groups=RG,
                ins=[x8i[:]], outs=[xga[:]])

            for t in range(NW):
                a_sb = wp.tile([8, WT], bf16, tag="attr")
                nc.sync.dma_start(a_sb[:], attrT[:, t * WT:(t + 1) * WT])
                ms = wp.tile([P, G], i32, tag="msrc")
                nc.sync.dma_start(ms[:], msrc[t])
                sv = wp.tile([P, G], bf16, tag="slotv")
                nc.sync.dma_start(sv[:], slotv[t])

                # one-hot segment matrix [edge, G*slot] built on device
                oh = wp.tile([P, WT], bf16, tag="oh")
                nc.vector.tensor_tensor(
                    out=oh[:].rearrange("p (g s) -> p g s", s=P),
                    in0=sv[:, :, None].to_broadcast([P, G, P]),
                    in1=iosb[:].rearrange("p (g s) -> p g s", s=P),
                    op=OP.is_equal)

                xg = wp.tile([P, G, 16], bf16, tag="xg")
                for g in range(G):
                    nc.gpsimd.indirect_dma_start(
                        out=xg[:, g, :], out_offset=None, in_=xga[:],
                        in_offset=bass.IndirectOffsetOnAxis(ap=ms[:, g:g + 1], axis=0))

                ps1 = pm.tile([100, WT], f32, tag="mlp")
                nc.tensor.matmul(ps1[:], lhsT=W1sb[:], rhs=a_sb[:], start=True, stop=True)
                h1 = wp.tile([100, WT], bf16, tag="h1")
                nc.scalar.activation(h1[:], ps1[:], AT.Relu, bias=b1sb[:, 0:1])
                ps2 = pm.tile([100, WT], f32, tag="mlp")
                nc.tensor.matmul(ps2[:], lhsT=W2sb[:], rhs=h1[:], start=True, stop=True)
                h2 = wp.tile([100, WT], bf16, tag="h2")
                nc.scalar.activation(h2[:], ps2[:], AT.Relu, bias=b2sb[:, 0:1])
                ps3 = pm.tile([101, WT], f32, tag="mlp")
                nc.tensor.matmul(ps3[:], lhsT=W3sb[:], rhs=h2[:], start=True, stop=True)
                h3 = wp.tile([101, WT], bf16, tag="h3")
                nc.scalar.activation(h3[:], ps3[:], AT.Relu, bias=b3sb[:, 0:1])

                mt = wp.tile([P, G, 16], bf16, tag="mt")
                ag = pa.tile([P, 16], f32, tag="agg")
                for g in range(G):
                    wps = pw.tile([P, 256], f32, tag="w")
                    nc.tensor.matmul(wps[:], lhsT=h3[:, g * P:(g + 1) * P],
                                     rhs=W4sb[:], start=True, stop=True)
                    pr = wp.tile([P, 256], bf16, tag="prod")
                    nc.vector.tensor_tensor(
                        out=pr[:].rearrange("p (o i) -> p o i", i=16),
                        in0=wps[:].rearrange("p (o i) -> p o i", i=16),
                        in1=xg[:, g, :][:, None, :].to_broadcast([P, 16, 16]),
                        op=OP.mult)
                    nc.vector.reduce_sum(
                        out=mt[:, g, :],
                        in_=pr[:].rearrange("p (o i) -> p o i", i=16), axis=AX.X)
                    nc.tensor.matmul(ag[:], lhsT=oh[:, g * P:(g + 1) * P],
                                     rhs=mt[:, g, :], start=(g == 0), stop=(g == G - 1))
                scat = wp.tile([P, 16], f32, tag="scat")
                nc.scalar.copy(scat[:], ag[:])
                nc.sync.dma_start(table[t * P:(t + 1) * P, :], scat[:])

            nc.gpsimd.collective_compute(
                "ReduceScatter", OP.add, replica_groups=RG,
                ins=[table[:]], outs=[rs_out[:]])

            # finalize: out = rs/cnt + x@root + bias  (node-major chunks)
            for c in range(NCHUNK):
                tg = wp.tile([P, 16], f32, tag="tagg")
                nc.sync.dma_start(tg[:], rs_out[c * P:(c + 1) * P, :])
                ts = wp.tile([P, 16], f32, tag="tsc")
                nc.vector.tensor_tensor(
                    out=ts[:], in0=tg[:],
                    in1=rcsb[:, c:c + 1].to_broadcast([P, 16]), op=OP.mult)
                rp = pa.tile([P, 16], f32, tag="agg")
                nc.tensor.matmul(rp[:], lhsT=xtsb[:, c * P:(c + 1) * P],
                                 rhs=rsb[:], start=True, stop=True)
                ot = wp.tile([P, 16], f32, tag="ot")
                nc.vector.tensor_tensor(out=ot[:], in0=rp[:], in1=ts[:], op=OP.add)
                nc.sync.dma_start(out[c * P:(c + 1) * P, :], ot[:])

    nc.compile()
    _PROG_CACHE[G] = nc
    return nc


# ------------------------------------------------------------------- driver

def _assemble(outs):
    """outs: [n_cores, NSLICE, 16] -> [N_NODES, 16] f32."""
    return np.concatenate(list(outs), axis=0)[:N_NODES].astype(np.float32)


def _run(inputs, trace=False):
    in_maps, G = _prep_inputs(**inputs)
    nc = build_program(G)
    res = run_bass_kernel_spmd(nc, in_maps, list(range(N_CORES)), trace=trace)
    out = _assemble([r["out"] for r in res.results])
    return out, res


def kernel(**inputs) -> np.ndarray:
    out, _ = _run(inputs, trace=False)
    return out
